# revision 67
# baseline (speedup 1.0000x reference)
"""Trainium2 Bass kernel for MHA (B=4, L=2048, D=1024, H=16, causal mask).

Sharding: 8 cores = (batch b, head-group g) with b = core//2, g = core%2.
Each core computes heads [g*8, (g+1)*8) for batch b and produces a partial
O-projection output [L, D] (bf16); the host sums the two head-group
partials per batch in fp32 and adds the output bias.

On-core dataflow:
  xT [c, q]    <- straight DMA of host-pre-transposed bf16(x[b].T)
  qT/kT        <- bf16 W-slice projection (lhsT=W tile, rhs=xT) into PSUM;
                  the DVE bias-add writes fp8e4m3, and a shuffle DMA
                  repacks [128,512] -> [32,2(head),2(ktile),512] for
                  DoubleRow (host permutes W q/k columns to match, see
                  QK_PERM)
  v [k, d]     <- bf16 projection (lhsT=xT tile, rhs=Wv) plus a ones
                  column per head (softmax row sums)
  scoresT [k, q] per (chunk, k-tile, head): ONE fp8 DoubleRow matmul
                  (K=2x32) at 0.5 cycles/row -- 2x faster than bf16
  attnT = exp(0.125 * scoresT) on ACT (bf16 out); mixed 128x128 blocks
  multiply by a 0/1 mask tile on GPSIMD; fully-masked blocks are skipped
  ctx [q, 65] per 128-q subtile: accumulate over k-tiles with
  lhsT=attnT block [128k, 128q], rhs=[V|1] [128k, 65]; col 64 = row sum.
  Eight (head, subtile) groups pack into one 2-bank PSUM tile at 128-col
  pitch; PSUM start=True lazily zeroes a whole 2KB bank, so each bank
  gets exactly one start and every group's first write lands on
  pending-zero bytes.
  normalize: per-partition reciprocal of col 64 + tensor_scalar multiply
  (DVE), then DMA-xbar-transpose [128q,128d]->[128d,128q] into per-sub
  ctxT tiles.
  out[q, m] = lhsT=ctxT block, rhs=Wo slice (bf16); DVE copy to bf16
  staging, qtile-pair out-DMAs.

Scheduling: chunk-major order (all pairs finish chunk c before c+1) so
O-projection work unlocks uniformly; projection/V/O chains interleave
as deficit-driven "fillers" into the ACT-paced attention loop so the PE
never starves while exp runs.  DMA queues are partitioned: latency-
critical loads/shuffles/transposes/stores on the SP HWDGE queue,
dependency-free bulk weight/x loads on the Pool SWDGE queue.
"""

import math
import sys
from collections import deque

import numpy as np

if "/opt/trn_rl_repo" not in sys.path:
    sys.path.insert(0, "/opt/trn_rl_repo")

import ml_dtypes  # noqa: E402

import concourse.bacc as bacc  # noqa: E402
import concourse.bass as bass  # noqa: E402
import concourse.mybir as mybir  # noqa: E402
import concourse.tile as tile  # noqa: E402
from concourse.bass_utils import run_bass_kernel_spmd  # noqa: E402

B, L, D = 4, 2048, 1024
H, DH = 16, 64
N_CORES = 8
HG = 2  # head groups (tensor parallel)
DG = D // HG  # 512 columns of QKV proj per core
HPC = H // HG  # 8 heads per core
PAIRS = HPC // 2  # 4 head pairs per core
CT = D // 128  # 8 contraction tiles for projections
QC, QW = 4, 512  # q chunks
QTN = L // 128  # 16 q subtiles
KTN, KW = L // 128, 128  # 16 k tiles
VW = 65  # V columns per head incl. ones column

F32 = mybir.dt.float32
BF16 = mybir.dt.bfloat16
FP8 = mybir.dt.float8e4
DR = mybir.MatmulPerfMode.DoubleRow
EXP = mybir.ActivationFunctionType.Exp
MUL = mybir.AluOpType.mult
ADD = mybir.AluOpType.add

# Within each pair's 128 projection columns, partition pi = 4*p2 + 2*h + t
# holds head-h dim d = 2*p2 + t.  The fp8 shuffle DMA [128,512] ->
# [32,2(h),2(t),512] then lands head h's 64 dims in partitions [0,32) at
# free dim t -- the layout DoubleRow needs.  Scores are order-invariant in
# d, so any consistent q/k permutation works.
QK_PERM = np.array([(((pi // 2) % 2) * 64 + 2 * (pi // 4) + (pi % 2))
                    for pi in range(128)])

_BUILD_CACHE: dict = {}


def _classify_mask(mask2d: np.ndarray):
    """mask2d: [L(q), L(k)] nonzero=keep. Per 128x128 (qtile, ktile) block:
    cls in {0: skip, 1: mixed, 2: keep-all}. Mixed blocks get a packed
    [128(k), 2, 128(q)] bf16 mask tile (duplicated for the two heads that
    share an attn tile); midx gives the unique-tile index."""
    keep = (mask2d != 0)
    cls = np.zeros((QTN, KTN), dtype=np.int64)
    tiles: dict[bytes, int] = {}
    packed: list[np.ndarray] = []
    midx = -np.ones((QTN, KTN), dtype=np.int64)
    for qt in range(QTN):
        for kt in range(KTN):
            blk = keep[qt * 128:(qt + 1) * 128, kt * KW:(kt + 1) * KW]
            if not blk.any():
                continue
            if blk.all():
                cls[qt, kt] = 2
                continue
            cls[qt, kt] = 1
            m = blk.T.astype(ml_dtypes.bfloat16)  # [128 k, 128 q]
            tl = np.stack([m, m], axis=1)  # [128, 2, 128]
            key = tl.tobytes()
            if key not in tiles:
                tiles[key] = len(packed)
                packed.append(tl)
            midx[qt, kt] = tiles[key]
    if packed:
        mask_arr = np.stack(packed)  # [n, 128, 2, 128]
    else:
        mask_arr = np.zeros((1, 128, 2, 128), dtype=ml_dtypes.bfloat16)
    return cls, midx, mask_arr


def _build(cls_key, n_mask_tiles):
    cls = np.asarray(cls_key[0]).reshape(QTN, KTN)
    midx = np.asarray(cls_key[1]).reshape(QTN, KTN)
    nt = max(1, n_mask_tiles)

    # per-qtile last contributing k-tile (normalize trigger)
    last_kt = [max((k for k in range(KTN) if cls[q, k] > 0), default=-1)
               for q in range(QTN)]

    nc = bacc.Bacc("TRN2", target_bir_lowering=False, debug=False,
                   num_devices=N_CORES)
    xt = nc.dram_tensor("xt", [D, L], BF16, kind="ExternalInput").ap()
    wqr = nc.dram_tensor("wqr", [PAIRS, 128, CT, 128], BF16,
                         kind="ExternalInput").ap()
    wkr = nc.dram_tensor("wkr", [PAIRS, 128, CT, 128], BF16,
                         kind="ExternalInput").ap()
    wvr = nc.dram_tensor("wvr", [128, CT, DG], BF16, kind="ExternalInput").ap()
    wor = nc.dram_tensor("wor", [128, PAIRS, D], BF16,
                         kind="ExternalInput").ap()
    bqv = nc.dram_tensor("bqv", [DG], F32, kind="ExternalInput").ap()
    bkv = nc.dram_tensor("bkv", [DG], F32, kind="ExternalInput").ap()
    bvt = nc.dram_tensor("bvt", [128, DG], F32, kind="ExternalInput").ap()
    mt = nc.dram_tensor("mt", [nt, 128, 2, 128], BF16,
                        kind="ExternalInput").ap()
    out = nc.dram_tensor("out", [L, D], BF16, kind="ExternalOutput").ap()

    with tile.TileContext(nc) as tc:
        with (
            tc.tile_pool(name="const", bufs=1) as cpool,
            tc.tile_pool(name="qk", bufs=8) as qkpool,
            tc.tile_pool(name="at", bufs=4) as apool,
            tc.tile_pool(name="cn", bufs=24) as cnpool,
            tc.tile_pool(name="rp", bufs=16) as rpool,
            tc.tile_pool(name="ctxT", bufs=PAIRS * QC * 4) as xpool,
            tc.tile_pool(name="ob", bufs=8) as opool,
            tc.tile_pool(name="pp", bufs=2, space="PSUM") as pp,
            tc.tile_pool(name="sp", bufs=2, space="PSUM") as sp,
            tc.tile_pool(name="cp", bufs=1, space="PSUM") as cp,
        ):
            # warm the ACT exp table before real work needs it
            wtile = cpool.tile([1, 8], F32, tag="warm")
            nc.gpsimd.memset(wtile[:], 0.0)
            nc.scalar.activation(wtile[:], wtile[:], EXP, scale=1.0)

            # ---- constant loads, priority-ordered for the startup path ----
            xTt = [[None] * 2 for _ in range(CT)]
            for ct in range(CT):
                for hf in range(2):
                    xTt[ct][hf] = cpool.tile([128, L // 2], BF16,
                                             tag=f"xT{ct}_{hf}",
                                             name=f"xT{ct}_{hf}")
            wqt = [cpool.tile([128, CT, 128], BF16, tag=f"wq{pr}",
                              name=f"wq{pr}") for pr in range(PAIRS)]
            wkt = [cpool.tile([128, CT, 128], BF16, tag=f"wk{pr}",
                              name=f"wk{pr}") for pr in range(PAIRS)]

            def ld(sb, dr):
                nc.sync.dma_start(sb, dr)

            def ldp(sb, dr):
                # bulk, dependency-free loads ride the otherwise-idle Pool
                # SWDGE queue so the SP HWDGE FIFO stays short for the
                # latency-critical q/k shuffles and ctxT transposes
                nc.gpsimd.dma_start(sb, dr)

            # critical path: first q/k chains need xT half-0 cols 0:512 +
            # wq0/wk0; biases/mask arrive while the chains run
            ld(wqt[0][:], wqr[0])
            ld(xTt[0][0][:, 0:QW], xt[0:128, 0:QW])
            ld(wkt[0][:], wkr[0])
            for ct in range(1, 4):
                ld(xTt[ct][0][:, 0:QW], xt[ct * 128:(ct + 1) * 128, 0:QW])
            ld(wqt[1][:], wqr[1])
            ld(wkt[1][:], wkr[1])
            for ct in range(4, CT):
                ld(xTt[ct][0][:, 0:QW], xt[ct * 128:(ct + 1) * 128, 0:QW])
            mk_sb = cpool.tile([128, nt, 2, 128], BF16, tag="mk")
            ld(mk_sb[:], mt.rearrange("n p a b -> p n a b"))
            bq_sb = cpool.tile([128, PAIRS], F32, tag="bq")
            ld(bq_sb[:], bqv.rearrange("(t p) -> p t", p=128))
            bk_sb = cpool.tile([128, PAIRS], F32, tag="bk")
            ld(bk_sb[:], bkv.rearrange("(t p) -> p t", p=128))
            wv_sb = cpool.tile([128, CT, DG], BF16, tag="wv")
            # SP queue: on Pool this 2.9us transfer is the bulk queue's head
            # and steals the shared DMA bus from the critical xT tiles
            ld(wv_sb[:], wvr[:])
            bv_sb = cpool.tile([128, DG], F32, tag="bv")
            ldp(bv_sb[:], bvt[:])
            for ct in range(CT):
                ldp(xTt[ct][0][:, QW:2 * QW],
                    xt[ct * 128:(ct + 1) * 128, QW:2 * QW])
            for ct in range(CT):
                ldp(xTt[ct][1][:], xt[ct * 128:(ct + 1) * 128, L // 2:L])
            wo_sb = cpool.tile([128, PAIRS, D], BF16, tag="wo")
            for pr in (2, 3):
                ldp(wqt[pr][:], wqr[pr])
                ldp(wkt[pr][:], wkr[pr])
            ldp(wo_sb[:], wor[:])

            # ---- registries ----
            qT: dict = {}  # (pair, chunk) -> [128, 512] bf16 (2 heads x 64d)
            kT: dict = {}  # (pair, kchunk) -> [128, 512]
            vv = [None] * KTN  # kt -> [128, HPC, VW] bf16
            ctxT = [[[None] * 4 for _ in range(QC)] for _ in range(PAIRS)]
            ob_t = [None] * QTN

            def shuffle_fp8(src8, tag, name, bufs):
                t2 = qkpool.tile([32, 2, 2, QW], FP8, tag=tag, name=name,
                                 bufs=bufs)
                nc.sync.dma_start(t2[:], src8[:])
                return t2

            def emit_q_chain(pr, qc):
                ps = pp.tile([128, QW], F32, tag="pp", name=f"psq{pr}_{qc}")
                for ct in range(CT):
                    nc.tensor.matmul(
                        ps[:], lhsT=wqt[pr][:, ct, :],
                        rhs=xTt[ct][qc // 2][:, (qc % 2) * QW:(qc % 2 + 1) * QW],
                        start=(ct == 0), stop=(ct == CT - 1))
                qt = qkpool.tile([128, QW], FP8, tag="qT",
                                 name=f"qT8{pr}_{qc}", bufs=6)
                nc.vector.tensor_scalar_add(qt[:], ps[:], bq_sb[:, pr:pr + 1])
                qT[(pr, qc)] = shuffle_fp8(qt, "qT2", f"qT2{pr}_{qc}", 6)

            def emit_k_chain(pr, kc):
                ps = pp.tile([128, QW], F32, tag="pp", name=f"psk{pr}_{kc}")
                for ct in range(CT):
                    nc.tensor.matmul(
                        ps[:], lhsT=wkt[pr][:, ct, :],
                        rhs=xTt[ct][kc // 2][:, (kc % 2) * QW:(kc % 2 + 1) * QW],
                        start=(ct == 0), stop=(ct == CT - 1))
                kt_ = qkpool.tile([128, QW], FP8, tag="kT",
                                  name=f"kT8{pr}_{kc}", bufs=10)
                nc.vector.tensor_scalar_add(kt_[:], ps[:], bk_sb[:, pr:pr + 1])
                kT[(pr, kc)] = shuffle_fp8(kt_, "kT2", f"kT2{pr}_{kc}", 18)

            def emit_v_chain(kt):
                ps = pp.tile([128, DG], F32, tag="pp", name=f"psv{kt}")
                for ct in range(CT):
                    nc.tensor.matmul(
                        ps[:],
                        lhsT=xTt[ct][kt // 8][:, (kt % 8) * 128:(kt % 8 + 1) * 128],
                        rhs=wv_sb[:, ct, :],
                        start=(ct == 0), stop=(ct == CT - 1))
                vt = cpool.tile([128, HPC, VW], BF16, tag=f"vv{kt}",
                                name=f"vv{kt}")
                nc.vector.tensor_tensor(
                    vt[:, :, 0:DH],
                    ps[:].rearrange("p (h d) -> p h d", d=DH),
                    bv_sb[:].rearrange("p (h d) -> p h d", d=DH),
                    ADD)
                nc.vector.memset(vt[:, :, DH:VW], 1.0)
                vv[kt] = vt

            pending_out: deque = deque()

            def flush_out(n=1):
                for _ in range(n):
                    if pending_out:
                        i = pending_out.popleft()
                        # qtile pair (i, i+1) shipped as one DMA
                        nc.sync.dma_start(
                            out[i * 128:(i + 2) * 128, :]
                            .rearrange("(j p) q -> p j q", p=128),
                            ob_t[i][:])

            def emit_o_half(i, mc):
                j = i // 4
                ip = i - (i % 2)
                if mc == 0 and i % 2 == 0:
                    ob_t[ip] = opool.tile([128, 2, D], BF16, tag="ob",
                                          name=f"ob{ip}")
                ob = ob_t[ip]
                po = pp.tile([128, QW], F32, tag="pp", name=f"po{i}_{mc}")
                for pr in range(PAIRS):
                    nc.tensor.matmul(
                        po[:], lhsT=ctxT[pr][j][i % 4][:],
                        rhs=wo_sb[:, pr, mc * QW:(mc + 1) * QW],
                        start=(pr == 0), stop=(pr == PAIRS - 1))
                nc.vector.tensor_copy(
                    ob[:, i % 2, mc * QW:(mc + 1) * QW], po[:])
                if i == QTN - 1:
                    # tail: ship each half as soon as its copy lands; qtile
                    # 14 (pair buddy) goes with the first half
                    if mc == 0:
                        nc.sync.dma_start(out[(i - 1) * 128:i * 128, :],
                                          ob[:, 0, :])
                    nc.sync.dma_start(
                        out[i * 128:(i + 1) * 128, mc * QW:(mc + 1) * QW],
                        ob[:, 1, mc * QW:(mc + 1) * QW])
                elif i % 2 == 1 and mc == 1:
                    # lag the out-DMA one O unit: emitted immediately it can
                    # hold SP.SEQ waiting on the DVE copy, starving queued
                    # ctxT transposes
                    flush_out()
                    pending_out.append(ip)

            # ---- filler scheduling ----
            # Deficit-driven: the attention inner loop is ACT(exp)-paced;
            # per iteration the PE has (act_cost - pe_cost) ns of slack that
            # filler chains (projections / V / O-proj) should fill.  Popping
            # only to cover measured deficit defers surplus work to the
            # filler-starved late pairs.
            PE_ROW = 1.0 / 2.4  # ns per output row at full clock
            ACT_EL = 1.0 / 1.2  # ns per free element
            ACT_OVH = 194.0
            UNIT_COST = {"q": 1707.0, "k": 1707.0, "v": 1707.0, "o": 854.0}

            fillers: deque = deque()
            staged: deque = deque()  # units whose deps landed too recently
            done_units: set = set()
            state = {"deficit": 0.0}

            def run_unit(u):
                kind = u[0]
                if kind == "q":
                    emit_q_chain(u[1], u[2])
                elif kind == "k":
                    emit_k_chain(u[1], u[2])
                elif kind == "v":
                    emit_v_chain(u[1])
                elif kind == "o":
                    # de-prioritize: the scheduler otherwise hoists O-proj
                    # matmuls ahead of ready attention work and the in-order
                    # PE stream stalls on the ctxT transpose latency
                    with tc.high_priority(offset=-1000000):
                        emit_o_half(u[1], u[2])
                done_units.add(u)

            def pop_by_deficit():
                while fillers and state["deficit"] >= UNIT_COST[fillers[0][0]]:
                    u = fillers.popleft()
                    run_unit(u)
                    state["deficit"] -= UNIT_COST[u[0]]

            def drain_until(unit):
                if unit in done_units:
                    return
                while fillers:
                    u = fillers.popleft()
                    run_unit(u)
                    state["deficit"] = 0.0
                    if u == unit:
                        return
                raise AssertionError(f"unit {unit} not in filler queue")

            # static filler order (availability-safe; drain_until enforces
            # correctness regardless of cadence)
            # chunk-major processing order: O-proj for chunk c unlocks after
            # its pair-3 pass, i.e. at c+1 quarters through the kernel, so
            # O filler work spreads uniformly instead of bunching at the end.
            chunk_seq = [(pr, c) for c in range(QC) for pr in range(PAIRS)]

            # qk units ahead of same-phase V units: the chunk-ahead prefetch
            # pulls qk without dragging V chains with it; V is force-drained
            # at its first ctx use anyway
            for pr in (1, 2, 3):
                fillers.append(("q", pr, 0))
                fillers.append(("k", pr, 0))
            for kt in range(0, 4):
                fillers.append(("v", kt))
            for c in range(1, QC):
                for pr in range(PAIRS):
                    fillers.append(("q", pr, c))
                    fillers.append(("k", pr, c))
                for kt in range(4 * c, 4 * c + 4):
                    fillers.append(("v", kt))

            # ---- prologue: pair-0 chunk-0 Q/K, per-ct interleaved ----
            psq0 = pp.tile([128, QW], F32, tag="pp", name="psq0_0")
            psk0 = pp.tile([128, QW], F32, tag="pp", name="psk0_0")
            for ct in range(CT):
                nc.tensor.matmul(psq0[:], lhsT=wqt[0][:, ct, :],
                                 rhs=xTt[ct][0][:, 0:QW],
                                 start=(ct == 0), stop=(ct == CT - 1))
                nc.tensor.matmul(psk0[:], lhsT=wkt[0][:, ct, :],
                                 rhs=xTt[ct][0][:, 0:QW],
                                 start=(ct == 0), stop=(ct == CT - 1))
            qt0 = qkpool.tile([128, QW], FP8, tag="qT", name="qT8_0_0",
                              bufs=6)
            # ACT is idle this early; keep the first bias adds off DVE's
            # critical chain
            nc.scalar.add(qt0[:], psq0[:], bq_sb[:, 0:1])
            qT[(0, 0)] = shuffle_fp8(qt0, "qT2", "qT2_0_0", 6)
            kt0 = qkpool.tile([128, QW], FP8, tag="kT", name="kT8_0_0",
                              bufs=10)
            nc.scalar.add(kt0[:], psk0[:], bk_sb[:, 0:1])
            kT[(0, 0)] = shuffle_fp8(kt0, "kT2", "kT2_0_0", 18)
            done_units.add(("q", 0, 0))
            done_units.add(("k", 0, 0))

            # ---- attention (chunk-major) ----
            for pr, c in chunk_seq:
                if True:
                    he = 2 * pr
                    drain_until(("q", pr, c))
                    drain_until(("k", pr, c))

                    cpt = cp.tile([128, 8, 128], F32, tag="cp",
                                  name=f"cp{pr}_{c}")
                    qth = qT[(pr, c)]
                    klist = [k for k in range(KTN)
                             if any(cls[4 * c + s, k] > 0 for s in range(4))]
                    # PSUM start=True lazily zeroes a whole 2KB bank, so each
                    # of the two banks of cpt gets exactly one start (first
                    # matmul) and one stop (last matmul); every group's first
                    # write still lands on pending-zero bytes -> overwrite.
                    bank_started = [False, False]
                    n_mm = sum(1 for k in klist for s in range(4)
                               if cls[4 * c + s, k] > 0)
                    mm_idx = 0
                    ci = chunk_seq.index((pr, c))
                    nxt = chunk_seq[ci + 1] if ci + 1 < len(chunk_seq) else None
                    for it_idx, kt in enumerate(klist):
                        if it_idx == 1 and nxt is not None:
                            # prefetch next chunk's q/k a full chunk early so
                            # the fp8 shuffle DMA latency hides completely
                            drain_until(("q",) + nxt)
                            drain_until(("k",) + nxt)
                        covered = [s for s in range(4)
                                   if cls[4 * c + s, kt] > 0]
                        qlo = 128 * covered[0]
                        w = 128 * (covered[-1] + 1) - qlo
                        kth = kT[(pr, kt // 4)]
                        kss = slice((kt % 4) * 128, (kt % 4 + 1) * 128)
                        st = sp.tile([128, 2, QW], F32, tag="sp")
                        for h in range(2):
                            nc.tensor.matmul(st[:, h, 0:w],
                                             lhsT=kth[:, h, :, kss],
                                             rhs=qth[:, h, :, qlo:qlo + w],
                                             start=True, stop=True,
                                             perf_mode=DR)
                        at = apool.tile([128, 2, QW], BF16, tag="attn")
                        nc.scalar.activation(at[:, :, 0:w], st[:, :, 0:w],
                                             EXP, scale=1.0 / math.sqrt(DH))
                        for s in covered:
                            if cls[4 * c + s, kt] == 1:
                                o = s * 128 - qlo
                                mi = int(midx[4 * c + s, kt])
                                # Pool engine: keeps DVE's in-order queue
                                # short for normalize muls and O-proj copies
                                nc.gpsimd.tensor_tensor(
                                    at[:, :, o:o + 128], at[:, :, o:o + 128],
                                    mk_sb[:, mi], MUL)
                        if pr == 0:
                            drain_until(("v", kt))
                        state["deficit"] += (2 * w * ACT_EL + ACT_OVH) - (
                            2 * w + 65 * 2 * len(covered)) * PE_ROW
                        pop_by_deficit()
                        # masked (diagonal) subtiles last: their ctx matmul
                        # waits on the DVE mask multiply
                        emit_order = ([s for s in covered
                                       if cls[4 * c + s, kt] == 2] +
                                      [s for s in covered
                                       if cls[4 * c + s, kt] == 1])
                        for s in emit_order:
                            o = s * 128 - qlo
                            mm_idx += 1
                            for h in range(2):
                                g = h * 4 + s
                                nc.tensor.matmul(
                                    cpt[:, g, 0:VW],
                                    lhsT=at[:, h, o:o + 128],
                                    rhs=vv[kt][:, he + h, :],
                                    start=not bank_started[h],
                                    stop=(mm_idx == n_mm),
                                    skip_group_check=True)
                                bank_started[h] = True
                        # normalize + transpose subtiles that just finished
                        for s in covered:
                            if last_kt[4 * c + s] != kt:
                                continue
                            cn = cnpool.tile([128, 128], BF16, tag="cn")
                            for h in range(2):
                                g = h * 4 + s
                                r = rpool.tile([128, 1], F32, tag="r")
                                nc.vector.reciprocal(r[:], cpt[:, g, 64:65])
                                nc.vector.tensor_scalar_mul(
                                    cn[:, h * 64:(h + 1) * 64],
                                    cpt[:, g, 0:DH], r[:])
                            cts = xpool.tile([128, 128], BF16,
                                             tag="ctxT",
                                             name=f"ctxT{pr}_{c}_{s}")
                            ctxT[pr][c][s] = cts
                            nc.sync.dma_start(cts[:], cn[:],
                                              transpose=True)
                            if pr == PAIRS - 1:
                                # one-normalize lag so the O-proj matmul
                                # doesn't stall on the transpose latency
                                fillers.extend(staged)
                                staged.clear()
                                staged.append(("o", 4 * c + s, 0))
                                staged.append(("o", 4 * c + s, 1))

            fillers.extend(staged)
            staged.clear()
            while fillers:
                run_unit(fillers.popleft())
            flush_out(len(pending_out))

    nc.compile()
    return nc


def _make_in_maps(x, attn_mask, Wq, bq, Wk, bk, Wv, bv, Wo, bo):
    """Shared between kernel() and test harnesses: returns (key, mask_arr,
    in_maps) for the 8 cores."""
    mask2d = np.broadcast_to(attn_mask, (1, 1, L, L))[0, 0]
    cls, midx, mask_arr = _classify_mask(mask2d)
    key = (cls.tobytes(), midx.tobytes(), mask_arr.shape[0])
    build_key = (tuple(cls.ravel()), tuple(midx.ravel()))

    # within each pair, reorder q/k projection columns for the fp8
    # DoubleRow score layout (see QK_PERM); biases use the same order
    perm_full = np.concatenate([pr * 128 + QK_PERM for pr in range(PAIRS)])

    in_maps = []
    for core in range(N_CORES):
        b, g = core // HG, core % HG
        gs = slice(g * DG, (g + 1) * DG)
        wq = Wq[:, gs][:, perm_full].astype(ml_dtypes.bfloat16)  # [D, DG]
        wk = Wk[:, gs][:, perm_full].astype(ml_dtypes.bfloat16)
        # wqt[pr][p, ct, d] must equal wq[ct*128 + p, pr*128 + d]
        wqr = np.ascontiguousarray(
            wq.reshape(CT, 128, PAIRS, 128).transpose(2, 1, 0, 3))
        wkr = np.ascontiguousarray(
            wk.reshape(CT, 128, PAIRS, 128).transpose(2, 1, 0, 3))
        # wv_sb[p, ct, d] = Wv[ct*128 + p, d]
        wvr = np.ascontiguousarray(
            Wv[:, gs].astype(ml_dtypes.bfloat16).reshape(CT, 128, DG)
            .transpose(1, 0, 2))
        # wo_sb[p, pr, m] = Wo[gs][pr*128 + p, m]
        wor = np.ascontiguousarray(
            Wo[gs, :].astype(ml_dtypes.bfloat16).reshape(PAIRS, 128, D)
            .transpose(1, 0, 2))
        in_maps.append({
            "xt": np.ascontiguousarray(
                x[b].T.astype(ml_dtypes.bfloat16)),
            "wqr": wqr,
            "wkr": wkr,
            "wvr": wvr,
            "wor": wor,
            "bqv": bq[gs][perm_full].copy(),
            "bkv": bk[gs][perm_full].copy(),
            "bvt": np.tile(bv[gs], (128, 1)),
            "mt": mask_arr,
        })
    return key, build_key, mask_arr, in_maps


def kernel(x, attn_mask, Wq, bq, Wk, bk, Wv, bv, Wo, bo):
    x = np.asarray(x, dtype=np.float32)
    attn_mask = np.asarray(attn_mask)
    Wq = np.asarray(Wq, dtype=np.float32)
    Wk = np.asarray(Wk, dtype=np.float32)
    Wv = np.asarray(Wv, dtype=np.float32)
    Wo = np.asarray(Wo, dtype=np.float32)
    bq = np.asarray(bq, dtype=np.float32)
    bk = np.asarray(bk, dtype=np.float32)
    bv = np.asarray(bv, dtype=np.float32)
    bo = np.asarray(bo, dtype=np.float32)

    key, build_key, mask_arr, in_maps = _make_in_maps(
        x, attn_mask, Wq, bq, Wk, bk, Wv, bv, Wo, bo)
    if key not in _BUILD_CACHE:
        _BUILD_CACHE[key] = _build(build_key, mask_arr.shape[0])
    nc = _BUILD_CACHE[key]

    res = run_bass_kernel_spmd(nc, in_maps, list(range(N_CORES)))
    out = np.empty((B, L, D), dtype=np.float32)
    for b in range(B):
        out[b] = (np.asarray(res.results[2 * b]["out"], dtype=np.float32)
                  + np.asarray(res.results[2 * b + 1]["out"],
                               dtype=np.float32) + bo)
    return out


# revision 68
# speedup vs baseline: 1.0346x; 1.0346x over previous
"""Trainium2 Bass kernel for MHA (B=4, L=2048, D=1024, H=16, causal mask).

Sharding: 8 cores = (batch b, head-group g) with b = core//2, g = core%2.
Each core computes heads [g*8, (g+1)*8) for batch b and produces a partial
O-projection output [L, D] (bf16); the host sums the two head-group
partials per batch in fp32 and adds the output bias.

On-core dataflow:
  xT [c, q]    <- straight DMA of host-pre-transposed bf16(x[b].T)
  qT/kT        <- bf16 W-slice projection (lhsT=W tile, rhs=xT) into PSUM;
                  the DVE bias-add writes fp8e4m3, and a shuffle DMA
                  repacks [128,512] -> [32,2(head),2(ktile),512] for
                  DoubleRow (host permutes W q/k columns to match, see
                  QK_PERM)
  v [k, d]     <- bf16 projection (lhsT=xT tile, rhs=Wv) plus a ones
                  column per head (softmax row sums)
  scoresT [k, q] per (chunk, k-tile, head): ONE fp8 DoubleRow matmul
                  (K=2x32) at 0.5 cycles/row -- 2x faster than bf16
  attnT = exp(0.125 * scoresT) on ACT (bf16 out); mixed 128x128 blocks
  multiply by a 0/1 mask tile on GPSIMD; fully-masked blocks are skipped
  ctx [q, 65] per 128-q subtile: accumulate over k-tiles with
  lhsT=attnT block [128k, 128q], rhs=[V|1] [128k, 65]; col 64 = row sum.
  Eight (head, subtile) groups pack into one 2-bank PSUM tile at 128-col
  pitch; PSUM start=True lazily zeroes a whole 2KB bank, so each bank
  gets exactly one start and every group's first write lands on
  pending-zero bytes.
  normalize: per-partition reciprocal of col 64 + tensor_scalar multiply
  (DVE), then DMA-xbar-transpose [128q,128d]->[128d,128q] into per-sub
  ctxT tiles.
  out[q, m] = lhsT=ctxT block, rhs=Wo slice (bf16); DVE copy to bf16
  staging, qtile-pair out-DMAs.

Scheduling: chunk-major order (all pairs finish chunk c before c+1) so
O-projection work unlocks uniformly; projection/V/O chains interleave
as deficit-driven "fillers" into the ACT-paced attention loop so the PE
never starves while exp runs.  DMA queues are partitioned: latency-
critical loads/shuffles/transposes/stores on the SP HWDGE queue,
dependency-free bulk weight/x loads on the Pool SWDGE queue.
"""

import math
import sys
from collections import deque

import numpy as np

if "/opt/trn_rl_repo" not in sys.path:
    sys.path.insert(0, "/opt/trn_rl_repo")

import ml_dtypes  # noqa: E402

import concourse.bacc as bacc  # noqa: E402
import concourse.bass as bass  # noqa: E402
import concourse.mybir as mybir  # noqa: E402
import concourse.tile as tile  # noqa: E402
from concourse.bass_utils import run_bass_kernel_spmd  # noqa: E402

B, L, D = 4, 2048, 1024
H, DH = 16, 64
N_CORES = 8
HG = 2  # head groups (tensor parallel)
DG = D // HG  # 512 columns of QKV proj per core
HPC = H // HG  # 8 heads per core
PAIRS = HPC // 2  # 4 head pairs per core
CT = D // 128  # 8 contraction tiles for projections
QC, QW = 4, 512  # q chunks
QTN = L // 128  # 16 q subtiles
KTN, KW = L // 128, 128  # 16 k tiles
VW = 65  # V columns per head incl. ones column

F32 = mybir.dt.float32
BF16 = mybir.dt.bfloat16
FP8 = mybir.dt.float8e4
DR = mybir.MatmulPerfMode.DoubleRow
EXP = mybir.ActivationFunctionType.Exp
MUL = mybir.AluOpType.mult
ADD = mybir.AluOpType.add

# Within each pair's 128 projection columns, partition pi = 4*p2 + 2*h + t
# holds head-h dim d = 2*p2 + t.  The fp8 shuffle DMA [128,512] ->
# [32,2(h),2(t),512] then lands head h's 64 dims in partitions [0,32) at
# free dim t -- the layout DoubleRow needs.  Scores are order-invariant in
# d, so any consistent q/k permutation works.
QK_PERM = np.array([(((pi // 2) % 2) * 64 + 2 * (pi // 4) + (pi % 2))
                    for pi in range(128)])

_BUILD_CACHE: dict = {}


def _classify_mask(mask2d: np.ndarray):
    """mask2d: [L(q), L(k)] nonzero=keep. Per 128x128 (qtile, ktile) block:
    cls in {0: skip, 1: mixed, 2: keep-all}. Mixed blocks get a packed
    [128(k), 2, 128(q)] bf16 mask tile (duplicated for the two heads that
    share an attn tile); midx gives the unique-tile index."""
    keep = (mask2d != 0)
    cls = np.zeros((QTN, KTN), dtype=np.int64)
    tiles: dict[bytes, int] = {}
    packed: list[np.ndarray] = []
    midx = -np.ones((QTN, KTN), dtype=np.int64)
    for qt in range(QTN):
        for kt in range(KTN):
            blk = keep[qt * 128:(qt + 1) * 128, kt * KW:(kt + 1) * KW]
            if not blk.any():
                continue
            if blk.all():
                cls[qt, kt] = 2
                continue
            cls[qt, kt] = 1
            m = blk.T.astype(ml_dtypes.bfloat16)  # [128 k, 128 q]
            tl = np.stack([m, m], axis=1)  # [128, 2, 128]
            key = tl.tobytes()
            if key not in tiles:
                tiles[key] = len(packed)
                packed.append(tl)
            midx[qt, kt] = tiles[key]
    if packed:
        mask_arr = np.stack(packed)  # [n, 128, 2, 128]
    else:
        mask_arr = np.zeros((1, 128, 2, 128), dtype=ml_dtypes.bfloat16)
    return cls, midx, mask_arr


def _build(cls_key, n_mask_tiles):
    cls = np.asarray(cls_key[0]).reshape(QTN, KTN)
    midx = np.asarray(cls_key[1]).reshape(QTN, KTN)
    nt = max(1, n_mask_tiles)

    # per-qtile last contributing k-tile (normalize trigger)
    last_kt = [max((k for k in range(KTN) if cls[q, k] > 0), default=-1)
               for q in range(QTN)]

    nc = bacc.Bacc("TRN2", target_bir_lowering=False, debug=False,
                   num_devices=N_CORES)
    xt = nc.dram_tensor("xt", [D, L], BF16, kind="ExternalInput").ap()
    wqr = nc.dram_tensor("wqr", [PAIRS, 128, CT, 128], BF16,
                         kind="ExternalInput").ap()
    wkr = nc.dram_tensor("wkr", [PAIRS, 128, CT, 128], BF16,
                         kind="ExternalInput").ap()
    wvr = nc.dram_tensor("wvr", [128, CT, DG], BF16, kind="ExternalInput").ap()
    wor = nc.dram_tensor("wor", [128, PAIRS, D], BF16,
                         kind="ExternalInput").ap()
    bqv = nc.dram_tensor("bqv", [DG], F32, kind="ExternalInput").ap()
    bkv = nc.dram_tensor("bkv", [DG], F32, kind="ExternalInput").ap()
    bvt = nc.dram_tensor("bvt", [128, DG], F32, kind="ExternalInput").ap()
    mt = nc.dram_tensor("mt", [nt, 128, 2, 128], BF16,
                        kind="ExternalInput").ap()
    out = nc.dram_tensor("out", [L, D], BF16, kind="ExternalOutput").ap()

    with tile.TileContext(nc) as tc:
        with (
            tc.tile_pool(name="const", bufs=1) as cpool,
            tc.tile_pool(name="qk", bufs=8) as qkpool,
            tc.tile_pool(name="at", bufs=4) as apool,
            tc.tile_pool(name="cn", bufs=24) as cnpool,
            tc.tile_pool(name="rp", bufs=16) as rpool,
            tc.tile_pool(name="ctxT", bufs=PAIRS * QC * 4) as xpool,
            tc.tile_pool(name="ob", bufs=8) as opool,
            tc.tile_pool(name="pp", bufs=2, space="PSUM") as pp,
            tc.tile_pool(name="sp", bufs=2, space="PSUM") as sp,
            tc.tile_pool(name="cp", bufs=1, space="PSUM") as cp,
        ):
            # warm the ACT exp table before real work needs it
            wtile = cpool.tile([1, 8], F32, tag="warm")
            nc.gpsimd.memset(wtile[:], 0.0)
            nc.scalar.activation(wtile[:], wtile[:], EXP, scale=1.0)

            # ---- constant loads, priority-ordered for the startup path ----
            xTt = [[None] * 2 for _ in range(CT)]
            for ct in range(CT):
                for hf in range(2):
                    xTt[ct][hf] = cpool.tile([128, L // 2], BF16,
                                             tag=f"xT{ct}_{hf}",
                                             name=f"xT{ct}_{hf}")
            wqt = [cpool.tile([128, CT, 128], BF16, tag=f"wq{pr}",
                              name=f"wq{pr}") for pr in range(PAIRS)]
            wkt = [cpool.tile([128, CT, 128], BF16, tag=f"wk{pr}",
                              name=f"wk{pr}") for pr in range(PAIRS)]

            def ld(sb, dr):
                nc.sync.dma_start(sb, dr)

            def ldp(sb, dr):
                # bulk, dependency-free loads ride the otherwise-idle Pool
                # SWDGE queue so the SP HWDGE FIFO stays short for the
                # latency-critical q/k shuffles and ctxT transposes
                nc.gpsimd.dma_start(sb, dr)

            # critical path: first q/k chains need xT half-0 cols 0:512 +
            # wq0/wk0; biases/mask arrive while the chains run
            ld(wqt[0][:], wqr[0])
            ld(xTt[0][0][:, 0:QW], xt[0:128, 0:QW])
            ld(wkt[0][:], wkr[0])
            for ct in range(1, 4):
                ld(xTt[ct][0][:, 0:QW], xt[ct * 128:(ct + 1) * 128, 0:QW])
            ld(wqt[1][:], wqr[1])
            ld(wkt[1][:], wkr[1])
            for ct in range(4, CT):
                ld(xTt[ct][0][:, 0:QW], xt[ct * 128:(ct + 1) * 128, 0:QW])
            mk_sb = cpool.tile([128, nt, 2, 128], BF16, tag="mk")
            ld(mk_sb[:], mt.rearrange("n p a b -> p n a b"))
            bq_sb = cpool.tile([128, PAIRS], F32, tag="bq")
            ld(bq_sb[:], bqv.rearrange("(t p) -> p t", p=128))
            bk_sb = cpool.tile([128, PAIRS], F32, tag="bk")
            ld(bk_sb[:], bkv.rearrange("(t p) -> p t", p=128))
            wv_sb = cpool.tile([128, CT, DG], BF16, tag="wv")
            ldp(wv_sb[:], wvr[:])
            bv_sb = cpool.tile([128, DG], F32, tag="bv")
            ldp(bv_sb[:], bvt[:])
            for ct in range(CT):
                ldp(xTt[ct][0][:, QW:2 * QW],
                    xt[ct * 128:(ct + 1) * 128, QW:2 * QW])
            for ct in range(CT):
                ldp(xTt[ct][1][:], xt[ct * 128:(ct + 1) * 128, L // 2:L])
            wo_sb = cpool.tile([128, PAIRS, D], BF16, tag="wo")
            for pr in (2, 3):
                ldp(wqt[pr][:], wqr[pr])
                ldp(wkt[pr][:], wkr[pr])
            ldp(wo_sb[:], wor[:])

            # ---- registries ----
            qT: dict = {}  # (pair, chunk) -> [128, 512] bf16 (2 heads x 64d)
            kT: dict = {}  # (pair, kchunk) -> [128, 512]
            vv = [None] * KTN  # kt -> [128, HPC, VW] bf16
            ctxT = [[[None] * 4 for _ in range(QC)] for _ in range(PAIRS)]
            ob_t = [None] * QTN

            def shuffle_fp8(src8, tag, name, bufs):
                t2 = qkpool.tile([32, 2, 2, QW], FP8, tag=tag, name=name,
                                 bufs=bufs)
                nc.sync.dma_start(t2[:], src8[:])
                return t2

            def emit_q_chain(pr, qc):
                ps = pp.tile([128, QW], F32, tag="pp", name=f"psq{pr}_{qc}")
                for ct in range(CT):
                    nc.tensor.matmul(
                        ps[:], lhsT=wqt[pr][:, ct, :],
                        rhs=xTt[ct][qc // 2][:, (qc % 2) * QW:(qc % 2 + 1) * QW],
                        start=(ct == 0), stop=(ct == CT - 1))
                qt = qkpool.tile([128, QW], FP8, tag="qT",
                                 name=f"qT8{pr}_{qc}", bufs=6)
                nc.vector.tensor_scalar_add(qt[:], ps[:], bq_sb[:, pr:pr + 1])
                qT[(pr, qc)] = shuffle_fp8(qt, "qT2", f"qT2{pr}_{qc}", 6)

            def emit_k_chain(pr, kc):
                ps = pp.tile([128, QW], F32, tag="pp", name=f"psk{pr}_{kc}")
                for ct in range(CT):
                    nc.tensor.matmul(
                        ps[:], lhsT=wkt[pr][:, ct, :],
                        rhs=xTt[ct][kc // 2][:, (kc % 2) * QW:(kc % 2 + 1) * QW],
                        start=(ct == 0), stop=(ct == CT - 1))
                kt_ = qkpool.tile([128, QW], FP8, tag="kT",
                                  name=f"kT8{pr}_{kc}", bufs=10)
                nc.vector.tensor_scalar_add(kt_[:], ps[:], bk_sb[:, pr:pr + 1])
                kT[(pr, kc)] = shuffle_fp8(kt_, "kT2", f"kT2{pr}_{kc}", 18)

            def emit_v_chain(kt):
                ps = pp.tile([128, DG], F32, tag="pp", name=f"psv{kt}")
                for ct in range(CT):
                    nc.tensor.matmul(
                        ps[:],
                        lhsT=xTt[ct][kt // 8][:, (kt % 8) * 128:(kt % 8 + 1) * 128],
                        rhs=wv_sb[:, ct, :],
                        start=(ct == 0), stop=(ct == CT - 1))
                vt = cpool.tile([128, HPC, VW], BF16, tag=f"vv{kt}",
                                name=f"vv{kt}")
                nc.vector.tensor_tensor(
                    vt[:, :, 0:DH],
                    ps[:].rearrange("p (h d) -> p h d", d=DH),
                    bv_sb[:].rearrange("p (h d) -> p h d", d=DH),
                    ADD)
                nc.vector.memset(vt[:, :, DH:VW], 1.0)
                vv[kt] = vt

            pending_out: deque = deque()

            def flush_out(n=1):
                for _ in range(n):
                    if pending_out:
                        i = pending_out.popleft()
                        # qtile pair (i, i+1) shipped as one DMA
                        nc.sync.dma_start(
                            out[i * 128:(i + 2) * 128, :]
                            .rearrange("(j p) q -> p j q", p=128),
                            ob_t[i][:])

            def emit_o_half(i, mc):
                j = i // 4
                ip = i - (i % 2)
                if mc == 0 and i % 2 == 0:
                    ob_t[ip] = opool.tile([128, 2, D], BF16, tag="ob",
                                          name=f"ob{ip}")
                ob = ob_t[ip]
                po = pp.tile([128, QW], F32, tag="pp", name=f"po{i}_{mc}")
                for pr in range(PAIRS):
                    nc.tensor.matmul(
                        po[:], lhsT=ctxT[pr][j][i % 4][:],
                        rhs=wo_sb[:, pr, mc * QW:(mc + 1) * QW],
                        start=(pr == 0), stop=(pr == PAIRS - 1))
                nc.vector.tensor_copy(
                    ob[:, i % 2, mc * QW:(mc + 1) * QW], po[:])
                if i == QTN - 1:
                    # tail: ship each half as soon as its copy lands; qtile
                    # 14 (pair buddy) goes with the first half
                    if mc == 0:
                        nc.sync.dma_start(out[(i - 1) * 128:i * 128, :],
                                          ob[:, 0, :])
                    nc.sync.dma_start(
                        out[i * 128:(i + 1) * 128, mc * QW:(mc + 1) * QW],
                        ob[:, 1, mc * QW:(mc + 1) * QW])
                elif i % 2 == 1 and mc == 1:
                    # lag the out-DMA one O unit: emitted immediately it can
                    # hold SP.SEQ waiting on the DVE copy, starving queued
                    # ctxT transposes
                    flush_out()
                    pending_out.append(ip)

            # ---- filler scheduling ----
            # Deficit-driven: the attention inner loop is ACT(exp)-paced;
            # per iteration the PE has (act_cost - pe_cost) ns of slack that
            # filler chains (projections / V / O-proj) should fill.  Popping
            # only to cover measured deficit defers surplus work to the
            # filler-starved late pairs.
            PE_ROW = 1.0 / 2.4  # ns per output row at full clock
            ACT_EL = 1.0 / 1.2  # ns per free element
            ACT_OVH = 194.0
            UNIT_COST = {"q": 1707.0, "k": 1707.0, "v": 1707.0, "o": 854.0}

            fillers: deque = deque()
            staged: deque = deque()  # units whose deps landed too recently
            done_units: set = set()
            state = {"deficit": 0.0}

            def run_unit(u):
                kind = u[0]
                if kind == "q":
                    emit_q_chain(u[1], u[2])
                elif kind == "k":
                    emit_k_chain(u[1], u[2])
                elif kind == "v":
                    emit_v_chain(u[1])
                elif kind == "o":
                    # de-prioritize: the scheduler otherwise hoists O-proj
                    # matmuls ahead of ready attention work and the in-order
                    # PE stream stalls on the ctxT transpose latency
                    with tc.high_priority(offset=-1000000):
                        emit_o_half(u[1], u[2])
                done_units.add(u)

            def pop_by_deficit():
                while fillers and state["deficit"] >= UNIT_COST[fillers[0][0]]:
                    u = fillers.popleft()
                    run_unit(u)
                    state["deficit"] -= UNIT_COST[u[0]]

            def drain_until(unit):
                if unit in done_units:
                    return
                while fillers:
                    u = fillers.popleft()
                    run_unit(u)
                    state["deficit"] = 0.0
                    if u == unit:
                        return
                raise AssertionError(f"unit {unit} not in filler queue")

            # static filler order (availability-safe; drain_until enforces
            # correctness regardless of cadence)
            # chunk-major processing order: O-proj for chunk c unlocks after
            # its pair-3 pass, i.e. at c+1 quarters through the kernel, so
            # O filler work spreads uniformly instead of bunching at the end.
            chunk_seq = [(pr, c) for c in range(QC) for pr in range(PAIRS)]

            # qk units ahead of same-phase V units: the chunk-ahead prefetch
            # pulls qk without dragging V chains with it; V is force-drained
            # at its first ctx use anyway
            for pr in (1, 2, 3):
                fillers.append(("q", pr, 0))
                fillers.append(("k", pr, 0))
            for kt in range(0, 4):
                fillers.append(("v", kt))
            for c in range(1, QC):
                for pr in range(PAIRS):
                    fillers.append(("q", pr, c))
                    fillers.append(("k", pr, c))
                for kt in range(4 * c, 4 * c + 4):
                    fillers.append(("v", kt))

            # ---- prologue: pair-0 chunk-0 Q/K, per-ct interleaved ----
            psq0 = pp.tile([128, QW], F32, tag="pp", name="psq0_0")
            psk0 = pp.tile([128, QW], F32, tag="pp", name="psk0_0")
            for ct in range(CT):
                nc.tensor.matmul(psq0[:], lhsT=wqt[0][:, ct, :],
                                 rhs=xTt[ct][0][:, 0:QW],
                                 start=(ct == 0), stop=(ct == CT - 1))
                nc.tensor.matmul(psk0[:], lhsT=wkt[0][:, ct, :],
                                 rhs=xTt[ct][0][:, 0:QW],
                                 start=(ct == 0), stop=(ct == CT - 1))
            qt0 = qkpool.tile([128, QW], FP8, tag="qT", name="qT8_0_0",
                              bufs=6)
            # ACT is idle this early; keep the first bias adds off DVE's
            # critical chain
            nc.scalar.add(qt0[:], psq0[:], bq_sb[:, 0:1])
            qT[(0, 0)] = shuffle_fp8(qt0, "qT2", "qT2_0_0", 6)
            kt0 = qkpool.tile([128, QW], FP8, tag="kT", name="kT8_0_0",
                              bufs=10)
            nc.scalar.add(kt0[:], psk0[:], bk_sb[:, 0:1])
            kT[(0, 0)] = shuffle_fp8(kt0, "kT2", "kT2_0_0", 18)
            done_units.add(("q", 0, 0))
            done_units.add(("k", 0, 0))

            # ---- attention (chunk-major) ----
            for pr, c in chunk_seq:
                if True:
                    he = 2 * pr
                    drain_until(("q", pr, c))
                    drain_until(("k", pr, c))

                    cpt = cp.tile([128, 8, 128], F32, tag="cp",
                                  name=f"cp{pr}_{c}")
                    qth = qT[(pr, c)]
                    klist = [k for k in range(KTN)
                             if any(cls[4 * c + s, k] > 0 for s in range(4))]
                    # PSUM start=True lazily zeroes a whole 2KB bank, so each
                    # of the two banks of cpt gets exactly one start (first
                    # matmul) and one stop (last matmul); every group's first
                    # write still lands on pending-zero bytes -> overwrite.
                    bank_started = [False, False]
                    n_mm = sum(1 for k in klist for s in range(4)
                               if cls[4 * c + s, k] > 0)
                    mm_idx = 0
                    ci = chunk_seq.index((pr, c))
                    nxt = chunk_seq[ci + 1] if ci + 1 < len(chunk_seq) else None
                    for it_idx, kt in enumerate(klist):
                        if it_idx == 1 and nxt is not None:
                            # prefetch next chunk's q/k a full chunk early so
                            # the fp8 shuffle DMA latency hides completely
                            drain_until(("q",) + nxt)
                            drain_until(("k",) + nxt)
                        covered = [s for s in range(4)
                                   if cls[4 * c + s, kt] > 0]
                        qlo = 128 * covered[0]
                        w = 128 * (covered[-1] + 1) - qlo
                        kth = kT[(pr, kt // 4)]
                        kss = slice((kt % 4) * 128, (kt % 4 + 1) * 128)
                        st = sp.tile([128, 2, QW], F32, tag="sp")
                        for h in range(2):
                            nc.tensor.matmul(st[:, h, 0:w],
                                             lhsT=kth[:, h, :, kss],
                                             rhs=qth[:, h, :, qlo:qlo + w],
                                             start=True, stop=True,
                                             perf_mode=DR)
                        at = apool.tile([128, 2, QW], BF16, tag="attn")
                        nc.scalar.activation(at[:, :, 0:w], st[:, :, 0:w],
                                             EXP, scale=1.0 / math.sqrt(DH))
                        for s in covered:
                            if cls[4 * c + s, kt] == 1:
                                o = s * 128 - qlo
                                mi = int(midx[4 * c + s, kt])
                                # Pool engine: keeps DVE's in-order queue
                                # short for normalize muls and O-proj copies
                                nc.gpsimd.tensor_tensor(
                                    at[:, :, o:o + 128], at[:, :, o:o + 128],
                                    mk_sb[:, mi], MUL)
                        if pr == 0:
                            drain_until(("v", kt))
                        state["deficit"] += (2 * w * ACT_EL + ACT_OVH) - (
                            2 * w + 65 * 2 * len(covered)) * PE_ROW
                        pop_by_deficit()
                        # masked (diagonal) subtiles last: their ctx matmul
                        # waits on the DVE mask multiply
                        emit_order = ([s for s in covered
                                       if cls[4 * c + s, kt] == 2] +
                                      [s for s in covered
                                       if cls[4 * c + s, kt] == 1])
                        for s in emit_order:
                            o = s * 128 - qlo
                            mm_idx += 1
                            for h in range(2):
                                g = h * 4 + s
                                nc.tensor.matmul(
                                    cpt[:, g, 0:VW],
                                    lhsT=at[:, h, o:o + 128],
                                    rhs=vv[kt][:, he + h, :],
                                    start=not bank_started[h],
                                    stop=(mm_idx == n_mm),
                                    skip_group_check=True)
                                bank_started[h] = True
                        # normalize + transpose subtiles that just finished
                        for s in covered:
                            if last_kt[4 * c + s] != kt:
                                continue
                            cn = cnpool.tile([128, 128], BF16, tag="cn")
                            for h in range(2):
                                g = h * 4 + s
                                r = rpool.tile([128, 1], F32, tag="r")
                                nc.vector.reciprocal(r[:], cpt[:, g, 64:65])
                                nc.vector.tensor_scalar_mul(
                                    cn[:, h * 64:(h + 1) * 64],
                                    cpt[:, g, 0:DH], r[:])
                            cts = xpool.tile([128, 128], BF16,
                                             tag="ctxT",
                                             name=f"ctxT{pr}_{c}_{s}")
                            ctxT[pr][c][s] = cts
                            nc.sync.dma_start(cts[:], cn[:],
                                              transpose=True)
                            if pr == PAIRS - 1:
                                # one-normalize lag so the O-proj matmul
                                # doesn't stall on the transpose latency
                                fillers.extend(staged)
                                staged.clear()
                                staged.append(("o", 4 * c + s, 0))
                                staged.append(("o", 4 * c + s, 1))

            fillers.extend(staged)
            staged.clear()
            while fillers:
                run_unit(fillers.popleft())
            flush_out(len(pending_out))

    nc.compile()
    return nc


def _make_in_maps(x, attn_mask, Wq, bq, Wk, bk, Wv, bv, Wo, bo):
    """Shared between kernel() and test harnesses: returns (key, mask_arr,
    in_maps) for the 8 cores."""
    mask2d = np.broadcast_to(attn_mask, (1, 1, L, L))[0, 0]
    cls, midx, mask_arr = _classify_mask(mask2d)
    key = (cls.tobytes(), midx.tobytes(), mask_arr.shape[0])
    build_key = (tuple(cls.ravel()), tuple(midx.ravel()))

    # within each pair, reorder q/k projection columns for the fp8
    # DoubleRow score layout (see QK_PERM); biases use the same order
    perm_full = np.concatenate([pr * 128 + QK_PERM for pr in range(PAIRS)])

    in_maps = []
    for core in range(N_CORES):
        b, g = core // HG, core % HG
        gs = slice(g * DG, (g + 1) * DG)
        wq = Wq[:, gs][:, perm_full].astype(ml_dtypes.bfloat16)  # [D, DG]
        wk = Wk[:, gs][:, perm_full].astype(ml_dtypes.bfloat16)
        # wqt[pr][p, ct, d] must equal wq[ct*128 + p, pr*128 + d]
        wqr = np.ascontiguousarray(
            wq.reshape(CT, 128, PAIRS, 128).transpose(2, 1, 0, 3))
        wkr = np.ascontiguousarray(
            wk.reshape(CT, 128, PAIRS, 128).transpose(2, 1, 0, 3))
        # wv_sb[p, ct, d] = Wv[ct*128 + p, d]
        wvr = np.ascontiguousarray(
            Wv[:, gs].astype(ml_dtypes.bfloat16).reshape(CT, 128, DG)
            .transpose(1, 0, 2))
        # wo_sb[p, pr, m] = Wo[gs][pr*128 + p, m]
        wor = np.ascontiguousarray(
            Wo[gs, :].astype(ml_dtypes.bfloat16).reshape(PAIRS, 128, D)
            .transpose(1, 0, 2))
        in_maps.append({
            "xt": np.ascontiguousarray(
                x[b].T.astype(ml_dtypes.bfloat16)),
            "wqr": wqr,
            "wkr": wkr,
            "wvr": wvr,
            "wor": wor,
            "bqv": bq[gs][perm_full].copy(),
            "bkv": bk[gs][perm_full].copy(),
            "bvt": np.tile(bv[gs], (128, 1)),
            "mt": mask_arr,
        })
    return key, build_key, mask_arr, in_maps


def kernel(x, attn_mask, Wq, bq, Wk, bk, Wv, bv, Wo, bo):
    x = np.asarray(x, dtype=np.float32)
    attn_mask = np.asarray(attn_mask)
    Wq = np.asarray(Wq, dtype=np.float32)
    Wk = np.asarray(Wk, dtype=np.float32)
    Wv = np.asarray(Wv, dtype=np.float32)
    Wo = np.asarray(Wo, dtype=np.float32)
    bq = np.asarray(bq, dtype=np.float32)
    bk = np.asarray(bk, dtype=np.float32)
    bv = np.asarray(bv, dtype=np.float32)
    bo = np.asarray(bo, dtype=np.float32)

    key, build_key, mask_arr, in_maps = _make_in_maps(
        x, attn_mask, Wq, bq, Wk, bk, Wv, bv, Wo, bo)
    if key not in _BUILD_CACHE:
        _BUILD_CACHE[key] = _build(build_key, mask_arr.shape[0])
    nc = _BUILD_CACHE[key]

    res = run_bass_kernel_spmd(nc, in_maps, list(range(N_CORES)))
    out = np.empty((B, L, D), dtype=np.float32)
    for b in range(B):
        out[b] = (np.asarray(res.results[2 * b]["out"], dtype=np.float32)
                  + np.asarray(res.results[2 * b + 1]["out"],
                               dtype=np.float32) + bo)
    return out
